# revision 10
# baseline (speedup 1.0000x reference)
"""Trainium2 Bass kernel for a 3-layer GRU (PyTorch gate order) + final FC.

Problem shapes (hardcoded): x [256, 512, 64], H=128, 3 layers, NCLASS=6.
Sharding: data-parallel over batch, 32 rows per core on 8 cores.

Per-core design:
  - Partition layout [96, *]: rows 32l:32l+32 hold layer l's batch (32 rows).
    The 3 layers run software-pipelined with a 1-slot lag (layer l processes
    timestep t at slot s = t + l), so one set of batched elementwise
    instructions covers all three layers each slot.
  - PSUM gate layout per slot (one bank, 512 f32):
      cols 0:128   = gxn  (W_in x + b_in)
      cols 128:256 = r pre-act (W_ir x + W_hr h + b_ir + b_hr)
      cols 256:384 = z pre-act
      cols 384:512 = hn   (W_hn h + b_hn)
    Biases enter via a K=3 selector matmul (E[3,96] x B3[3,512]) that also
    opens the accumulation group; the input projection (lhsT = x_t^T or
    h^{l-1}_t{}^T) and recurrent matmuls accumulate on top. The three
    layers' recurrent matmuls are column-tiled (out base partitions
    0/32/64) and run concurrently in the PE array.
  - GRU cell: r,z = sigmoid(psum), u = 1-z, q = z*h, t = r*hn,
    npre = t+gxn, n = tanh(npre), h' = u*n + q.
  - h' [96,128] is transposed each slot via the PE (identity matmul) and
    copied to SBUF as hT [128,96]; hT feeds both the next slot's recurrent
    matmuls and the next layer's input projection.
"""

import numpy as np

B, T, IN, H, NCLASS = 256, 512, 64, 128, 6
NCORES = 8
BL = B // NCORES  # 32
G3 = 3 * H  # 384
NL = 3  # layers
P = NL * BL  # 96 partitions of batch x layer
WCOLS0 = 6 * G3 + 4 * H + 2 * P + 2 * NCLASS + BL  # packed const columns
WCOLS = WCOLS0 + T * BL  # + transposed input at rows 0:64, cols WCOLS0:

_cached = {}


def _build_bass():
    if "nc" in _cached:
        return _cached["nc"]

    from contextlib import ExitStack

    import concourse.bass as bass
    import concourse.tile as tile
    from concourse import mybir
    from concourse.tile_rust import add_dep_helper

    f32 = mybir.dt.float32
    AF = mybir.ActivationFunctionType
    ALU = mybir.AluOpType

    nc = bass.Bass()

    # ---- DRAM I/O (per core; weights identical across cores) ----
    # weights, constants AND the transposed input all packed into one
    # [128, WCOLS] tensor: one DMA -> one semaphore -> instructions stay
    # under the ISA's tiny per-instruction sync-wait limit
    wp_d = nc.dram_tensor("wpack", [128, WCOLS], f32, kind="ExternalInput")
    out_d = nc.dram_tensor("out", [BL, NCLASS], f32, kind="ExternalOutput")

    NSLOT = T + NL - 1  # 514

    with ExitStack() as ctx:
        tc = ctx.enter_context(tile.TileContext(nc))
        const = ctx.enter_context(tc.tile_pool(name="const", bufs=1))
        work = ctx.enter_context(tc.tile_pool(name="work", bufs=3))
        psum = ctx.enter_context(tc.tile_pool(name="psum", bufs=4, space="PSUM"))
        psum_t = ctx.enter_context(tc.tile_pool(name="psum_t", bufs=2, space="PSUM"))
        psum_fc = ctx.enter_context(tc.tile_pool(name="psum_fc", bufs=1, space="PSUM"))

        # ---- load constants ----
        wpack = const.tile([128, WCOLS], f32)
        wp_dma = nc.sync.dma_start(out=wpack, in_=wp_d[:, :])
        xT = wpack[0:IN, WCOLS0 : WCOLS0 + T * BL]
        rih = [
            wpack[0:IN, 0:G3],
            wpack[0:H, G3 : 2 * G3],
            wpack[0:H, 2 * G3 : 3 * G3],
        ]
        rhh = [wpack[0:H, (3 + l) * G3 : (4 + l) * G3] for l in range(NL)]
        c0 = 6 * G3
        b3 = wpack[0:NL, c0 : c0 + 4 * H]
        esel = wpack[0:NL, c0 + 4 * H : c0 + 4 * H + P]
        id96 = wpack[0:P, c0 + 4 * H + P : c0 + 4 * H + 2 * P]
        c1 = c0 + 4 * H + 2 * P
        fcw = wpack[0:H, c1 : c1 + NCLASS]
        fcb = wpack[0:1, c1 + NCLASS : c1 + 2 * NCLASS]
        ones = wpack[0:1, c1 + 2 * NCLASS : c1 + 2 * NCLASS + BL]

        # ---- persistent state rings ----
        hT_ring = [const.tile([H, P], f32, tag=f"hT{i}", name=f"hT{i}") for i in range(3)]
        hB_ring = [const.tile([P, H], f32, tag=f"hB{i}", name=f"hB{i}") for i in range(2)]
        for tl in hT_ring:
            nc.vector.memset(tl, 0.0)
        for tl in hB_ring:
            nc.vector.memset(tl, 0.0)

        tr_hist = []
        for s in range(NSLOT):
            a = max(0, s - (T - 1))  # first active layer
            b = min(NL - 1, s) + 1  # last active layer + 1
            pa, pb = 32 * a, 32 * b
            hT_prev = hT_ring[(s - 1) % 3]
            hB_prev = hB_ring[(s - 1) % 2]
            hB_cur = hB_ring[s % 2]

            ps = psum.tile([P, 4 * H], f32)

            # bias matmul opens the accumulation group (full partition range:
            # matmuls with out base partition 32 may span at most 32 rows)
            bias_mm = nc.tensor.matmul(
                ps[:, :],
                esel[:, :],
                b3[:, :],
                start=True,
                stop=False,
            )
            # keep the bias matmul from floating ahead of the transpose two
            # slots back: by then the PE has already waited on recent DVE/ACT
            # ticks, so psum-recycle deps are subsumed and the matmul stays
            # under the ISA's 2-sync-wait limit
            if len(tr_hist) >= 2:
                add_dep_helper(
                    bias_mm.ins, tr_hist[-2].ins, sync=False,
                    reason="cap matmul sync waits",
                )
            # input projections (cols 0:384 = gxn|r|z)
            for l in range(a, b):
                t_l = s - l
                if l == 0:
                    lhs = xT[:, t_l * BL : (t_l + 1) * BL]
                else:
                    lhs = hT_prev[:, 32 * (l - 1) : 32 * l]
                nc.tensor.matmul(
                    ps[32 * l : 32 * (l + 1), 0:G3],
                    lhs,
                    rih[l][:, :],
                    start=False,
                    stop=False,
                )
            # recurrent matmuls (cols 128:512 = r|z|hn)
            for l in range(a, b):
                nc.tensor.matmul(
                    ps[32 * l : 32 * (l + 1), H : 4 * H],
                    hT_prev[:, 32 * l : 32 * (l + 1)],
                    rhh[l][:, :],
                    start=False,
                    stop=(l == b - 1),
                )

            rz = work.tile([P, 2 * H], f32, tag="rz")
            u = work.tile([P, H], f32, tag="u")
            q = work.tile([P, H], f32, tag="q")
            tt = work.tile([P, H], f32, tag="tt")
            npre = work.tile([P, H], f32, tag="npre")
            n = work.tile([P, H], f32, tag="n")
            w = work.tile([P, H], f32, tag="w")
            # PSUM access patterns starting at partition 32 may span at most
            # 32 partitions -> split the [32:96] ramp slot into two ranges.
            rngs = [(32, 64), (64, 96)] if (pa, pb) == (32, 96) else [(pa, pb)]
            for ra, rb in rngs:
                nc.scalar.activation(rz[ra:rb, 0:H], ps[ra:rb, H : 2 * H], AF.Sigmoid)
                nc.scalar.activation(
                    rz[ra:rb, H : 2 * H], ps[ra:rb, 2 * H : 3 * H], AF.Sigmoid
                )
                nc.vector.tensor_scalar(
                    u[ra:rb, :], rz[ra:rb, H : 2 * H], -1.0, 1.0, ALU.mult, ALU.add
                )
                nc.vector.tensor_mul(
                    q[ra:rb, :], rz[ra:rb, H : 2 * H], hB_prev[ra:rb, :]
                )
                nc.vector.tensor_mul(
                    tt[ra:rb, :], rz[ra:rb, 0:H], ps[ra:rb, 3 * H : 4 * H]
                )
                nc.vector.tensor_add(npre[ra:rb, :], tt[ra:rb, :], ps[ra:rb, 0:H])
                nc.scalar.activation(n[ra:rb, :], npre[ra:rb, :], AF.Tanh)
                nc.vector.tensor_mul(w[ra:rb, :], u[ra:rb, :], n[ra:rb, :])
                last_dve = nc.vector.tensor_add(
                    hB_cur[ra:rb, :], w[ra:rb, :], q[ra:rb, :]
                )

            # transpose h' -> hT for next slot's matmuls
            pt = psum_t.tile([H, P], f32)
            tr = nc.tensor.transpose(pt[:, :], hB_cur[:, :], id96[:, :])
            tr_hist.append(tr)
            nc.scalar.activation(hT_ring[s % 3][:, :], pt[:, :], AF.Copy)

        # ---- FC head on layer 2's final h ----
        s_last = NSLOT - 1
        pfc = psum_fc.tile([BL, NCLASS], f32)
        nc.tensor.matmul(
            pfc[:, :],
            hT_ring[s_last % 3][:, 64:96],
            fcw[:, :],
            start=True,
            stop=False,
        )
        last_pe = nc.tensor.matmul(
            pfc[:, :], ones[:, :], fcb[:, :], start=False, stop=True
        )
        out_sb = const.tile([BL, NCLASS], f32)
        last_act = nc.scalar.activation(out_sb[:, :], pfc[:, :], AF.Copy)
        out_dma = nc.sync.dma_start(out=out_d[:, :], in_=out_sb)

        # funnel all engine tails through SP nops with <=2 sync deps each, so
        # the TileContext-exit Drain needs no more than the ISA wait limit
        for dep in (last_act, last_pe, last_dve, wp_dma, out_dma):
            fn = nc.sync.nop()
            add_dep_helper(fn.ins, dep.ins, sync=True, reason="tail funnel")

    _cached["nc"] = nc
    return nc


def _prep_weights(w_ih0, w_ih1, w_ih2, w_hh, b_ih, b_hh, fc_w, fc_b):
    f = np.float32
    w_ih = [np.asarray(w_ih0, f), np.asarray(w_ih1, f), np.asarray(w_ih2, f)]
    w_hh = np.asarray(w_hh, f)
    b_ih = np.asarray(b_ih, f)
    b_hh = np.asarray(b_hh, f)

    wp = np.zeros((128, WCOLS), f)
    for l in range(NL):
        wi = w_ih[l]
        k = wi.shape[1]
        # psum cols 0:384 = [gxn | r | z] -> [W_in^T, W_ir^T, W_iz^T]
        wp[0:k, l * G3 : (l + 1) * G3] = np.concatenate(
            [wi[2 * H : 3 * H].T, wi[0:H].T, wi[H : 2 * H].T], axis=1
        )
        wh = w_hh[l]
        # psum cols 128:512 = [r | z | hn] -> [W_hr^T, W_hz^T, W_hn^T]
        wp[0:H, (3 + l) * G3 : (4 + l) * G3] = np.concatenate(
            [wh[0:H].T, wh[H : 2 * H].T, wh[2 * H : 3 * H].T], axis=1
        )
    c0 = 6 * G3
    for l in range(NL):
        wp[l, c0 : c0 + H] = b_ih[l, 2 * H : 3 * H]  # b_in
        wp[l, c0 + H : c0 + 2 * H] = b_ih[l, 0:H] + b_hh[l, 0:H]  # r
        wp[l, c0 + 2 * H : c0 + 3 * H] = b_ih[l, H : 2 * H] + b_hh[l, H : 2 * H]  # z
        wp[l, c0 + 3 * H : c0 + 4 * H] = b_hh[l, 2 * H : 3 * H]  # b_hn
        wp[l, c0 + 4 * H + 32 * l : c0 + 4 * H + 32 * (l + 1)] = 1.0  # esel
    wp[0:P, c0 + 4 * H + P : c0 + 4 * H + 2 * P] = np.eye(P, dtype=f)
    c1 = c0 + 4 * H + 2 * P
    wp[0:H, c1 : c1 + NCLASS] = np.asarray(fc_w, f).T
    wp[0, c1 + NCLASS : c1 + 2 * NCLASS] = np.asarray(fc_b, f)
    wp[0, c1 + 2 * NCLASS : c1 + 2 * NCLASS + BL] = 1.0  # ones
    return {"wpack": wp}


def kernel(x, w_ih0, w_ih1, w_ih2, w_hh, b_ih, b_hh, fc_w, fc_b, **_ignored):
    from concourse.bass_utils import run_bass_kernel_spmd

    x = np.asarray(x, np.float32)
    shared = _prep_weights(w_ih0, w_ih1, w_ih2, w_hh, b_ih, b_hh, fc_w, fc_b)

    in_maps = []
    for c in range(NCORES):
        xc = x[c * BL : (c + 1) * BL]  # [32, 512, 64]
        wp = shared["wpack"].copy()
        wp[0:IN, WCOLS0:] = xc.transpose(2, 1, 0).reshape(IN, T * BL)
        in_maps.append({"wpack": wp})

    nc = _build_bass()
    res = run_bass_kernel_spmd(nc, in_maps, core_ids=list(range(NCORES)))
    out = np.concatenate([r["out"] for r in res.results], axis=0)
    return out.astype(np.float32)


if __name__ == "__main__":
    rng = np.random.default_rng(0)
    ins = {
        "x": rng.standard_normal((B, T, IN), dtype=np.float32),
        "w_ih0": rng.standard_normal((G3, IN), dtype=np.float32) * 0.05,
        "w_ih1": rng.standard_normal((G3, H), dtype=np.float32) * 0.05,
        "w_ih2": rng.standard_normal((G3, H), dtype=np.float32) * 0.05,
        "w_hh": rng.standard_normal((3, G3, H), dtype=np.float32) * 0.05,
        "b_ih": rng.standard_normal((3, G3), dtype=np.float32) * 0.05,
        "b_hh": rng.standard_normal((3, G3), dtype=np.float32) * 0.05,
        "fc_w": rng.standard_normal((NCLASS, H), dtype=np.float32) * 0.05,
        "fc_b": rng.standard_normal((NCLASS,), dtype=np.float32) * 0.05,
    }
    print(kernel(**ins)[:2])
